# revision 7
# baseline (speedup 1.0000x reference)
"""Trainium2 Bass kernel for batched self-attention + exact GELU + residual.

Reference computation (per batch b):
    K = x[b] @ Wk ; Q = x[b] @ Wq ; V = x[b] @ Wv          # [S, D]
    S_mat = Q @ K^T          (no 1/sqrt(d) scaling)        # [S, S]
    A = softmax(S_mat, axis=-1)
    out[b] = gelu_exact(A @ V) + x[b]

Sharding: 8 cores = 4 batches x 2 query-halves. Each core computes the
full K^T / V for its batch (recomputed on the paired core) and its own
2048-row query slab, so no collectives are needed.

Per-core pipeline (all staged through internal DRAM):
  1a. Q^T[i,q]  = Wq^T x^T          -> DRAM   (i on partitions)
  1b. K^T[i,k]  = Wk^T x^T          -> DRAM
  1c. V[k,i]    = x Wv              -> DRAM   (k on partitions)
  2.  per 128-row q-tile: S = Q K^T via PSUM, row-max (free-dim reduce),
      exp on ScalarE (bias = -max, accum_out = row sum), PE-transpose of
      the exp'd tile -> A^T staged to DRAM, 1/l kept in SBUF.
  3.  V resident in SBUF; out = A^T.T @ V accumulated in PSUM,
      epilogue = gelu(psum * (1/l)) on ScalarE + residual add + store.
"""

import os

import numpy as np


def _ensure_paths():
    try:
        import concourse.bass  # noqa: F401
    except ImportError:
        import sys

        for p in ("/opt/trn_rl_repo", "/root/.axon_site/_ro/trn_rl_repo"):
            if os.path.isdir(p) and p not in sys.path:
                sys.path.insert(0, p)


_ensure_paths()

from contextlib import ExitStack  # noqa: E402

import concourse.bass as bass  # noqa: E402
import concourse.bacc as bacc  # noqa: E402
import concourse.mybir as mybir  # noqa: E402
import concourse.tile as tile  # noqa: E402
from concourse.masks import make_identity  # noqa: E402

FP32 = mybir.dt.float32

P = 128
B = 4
S = 4096  # sequence length (keys per core)
D = 1024  # model dim == inner dim
QH = S // 2  # queries per core (2048)
N_CORES = 8

# Matmul compute dtype: float32r runs the PE at 1 cycle/row (vs 4 for
# plain float32) with near-fp32 accuracy.
MM_DT = mybir.dt.float32r
R = MM_DT


def _mm(nc, out, lhsT, rhs, start, stop):
    nc.tensor.matmul(out, lhsT, rhs, start=start, stop=stop)


def build_nc(use_gelu=True):
    """Build the per-core Bass program (same program on all 8 cores)."""
    nc = bacc.Bacc(None, target_bir_lowering=False)

    xT = nc.declare_dram_parameter("xT", [D, S], R, isOutput=False)
    xTq = nc.declare_dram_parameter("xTq", [D, QH], R, isOutput=False)
    xq = nc.declare_dram_parameter("xq", [QH, D], FP32, isOutput=False)
    wq = nc.declare_dram_parameter("wq", [D, D], R, isOutput=False)
    wk = nc.declare_dram_parameter("wk", [D, D], R, isOutput=False)
    wv = nc.declare_dram_parameter("wv", [D, D], R, isOutput=False)
    out = nc.declare_dram_parameter("out", [QH, D], FP32, isOutput=True)

    DT = D // P  # 8 d-tiles
    IT = D // P  # 8 i-tiles
    KT = S // P  # 32 k-tiles
    QT = QH // P  # 16 q-tiles
    KB = S // 512  # 8 key blocks of 512
    QB = QH // 512  # 4 query blocks of 512

    xT_v = xT.rearrange("(dt p) s -> p dt s", p=P)
    xTq_v = xTq.rearrange("(dt p) q -> p dt q", p=P)
    wq_v = wq.rearrange("(dt p) i -> p dt i", p=P)
    wk_v = wk.rearrange("(dt p) i -> p dt i", p=P)
    wv_v = wv.rearrange("(dt p) i -> p dt i", p=P)

    with tile.TileContext(nc) as tc, ExitStack() as ctx:
        dram = ctx.enter_context(tc.tile_pool(name="dram", bufs=1, space="DRAM"))
        qt_d = dram.tile([D, QH], R)  # Q^T  [i, q]
        kt_d = dram.tile([D, S], R)  # K^T  [i, k]
        v_d = dram.tile([S, D], R)  # V    [k, i]
        at_d = dram.tile([S, QH], R)  # A^T  [k, q]

        qt_dv = qt_d.rearrange("(it p) q -> p it q", p=P)
        kt_dv = kt_d.rearrange("(it p) s -> p it s", p=P)
        v_dv = v_d.rearrange("(kt p) i -> p kt i", p=P)
        at_dv = at_d.rearrange("(kt p) q -> p kt q", p=P)

        persist = ctx.enter_context(tc.tile_pool(name="persist", bufs=1))
        rl_all = persist.tile([P, QT], FP32)  # 1/rowsum per q-tile
        ident = persist.tile([P, P], FP32)
        make_identity(nc, ident[:])

        # ---------------- Phase 1a: Q^T -> qt_d ----------------
        with (
            tc.tile_pool(name="w1a", bufs=1) as wpool,
            tc.tile_pool(name="x1a", bufs=2) as xpool,
            tc.tile_pool(name="o1a", bufs=3) as opool,
            tc.tile_pool(name="ps1a", bufs=4, space="PSUM") as pspool,
        ):
            wq_sb = wpool.tile([P, DT, D], R)
            nc.sync.dma_start(wq_sb[:], wq_v[:])
            for qb in range(QB):
                xt_t = xpool.tile([P, DT, 512], R)
                nc.sync.dma_start(xt_t[:], xTq_v[:, :, qb * 512 : (qb + 1) * 512])
                for it in range(IT):
                    ps = pspool.tile([P, 512], FP32)
                    for dt_ in range(DT):
                        _mm(
                            nc,
                            ps[:],
                            wq_sb[:, dt_, it * P : (it + 1) * P],
                            xt_t[:, dt_, :],
                            start=(dt_ == 0),
                            stop=(dt_ == DT - 1),
                        )
                    ot = opool.tile([P, 512], R)
                    nc.any.tensor_copy(ot[:], ps[:])
                    nc.sync.dma_start(
                        qt_dv[:, it, qb * 512 : (qb + 1) * 512], ot[:]
                    )

        # ---------------- Phase 1b: K^T -> kt_d ----------------
        with (
            tc.tile_pool(name="w1b", bufs=1) as wpool,
            tc.tile_pool(name="x1b", bufs=2) as xpool,
            tc.tile_pool(name="o1b", bufs=3) as opool,
            tc.tile_pool(name="ps1b", bufs=4, space="PSUM") as pspool,
        ):
            wk_sb = wpool.tile([P, DT, D], R)
            nc.sync.dma_start(wk_sb[:], wk_v[:])
            for kb in range(KB):
                xt_t = xpool.tile([P, DT, 512], R)
                nc.sync.dma_start(xt_t[:], xT_v[:, :, kb * 512 : (kb + 1) * 512])
                for it in range(IT):
                    ps = pspool.tile([P, 512], FP32)
                    for dt_ in range(DT):
                        _mm(
                            nc,
                            ps[:],
                            wk_sb[:, dt_, it * P : (it + 1) * P],
                            xt_t[:, dt_, :],
                            start=(dt_ == 0),
                            stop=(dt_ == DT - 1),
                        )
                    ot = opool.tile([P, 512], R)
                    nc.any.tensor_copy(ot[:], ps[:])
                    nc.sync.dma_start(
                        kt_dv[:, it, kb * 512 : (kb + 1) * 512], ot[:]
                    )

        # ---------------- Phase 1c: V -> v_d ----------------
        with (
            tc.tile_pool(name="w1c", bufs=1) as wpool,
            tc.tile_pool(name="x1c", bufs=2) as xpool,
            tc.tile_pool(name="o1c", bufs=3) as opool,
            tc.tile_pool(name="ps1c", bufs=4, space="PSUM") as pspool,
        ):
            wv_sb = wpool.tile([P, DT, D], R)
            nc.sync.dma_start(wv_sb[:], wv_v[:])
            for kt_i in range(KT):
                xt_t = xpool.tile([P, DT, P], R)
                nc.sync.dma_start(xt_t[:], xT_v[:, :, kt_i * P : (kt_i + 1) * P])
                for ib in range(2):
                    ps = pspool.tile([P, 512], FP32)
                    for dt_ in range(DT):
                        _mm(
                            nc,
                            ps[:],
                            xt_t[:, dt_, :],
                            wv_sb[:, dt_, ib * 512 : (ib + 1) * 512],
                            start=(dt_ == 0),
                            stop=(dt_ == DT - 1),
                        )
                    ot = opool.tile([P, 512], R)
                    nc.any.tensor_copy(ot[:], ps[:])
                    nc.sync.dma_start(
                        v_dv[:, kt_i, ib * 512 : (ib + 1) * 512], ot[:]
                    )

        # ---------- Phase 2: S = QK^T, softmax, A^T -> at_d ----------
        with (
            tc.tile_pool(name="ktres", bufs=1) as ktpool,
            tc.tile_pool(name="qts", bufs=2) as qtpool,
            tc.tile_pool(name="srow", bufs=1) as spool,
            tc.tile_pool(name="atac", bufs=2) as atpool,
            tc.tile_pool(name="stat", bufs=2) as stpool,
            tc.tile_pool(name="ps2", bufs=4, space="PSUM") as pspool,
            tc.tile_pool(name="tps2", bufs=2, space="PSUM") as tppool,
        ):
            kt_sb = ktpool.tile([P, IT, S], R)
            for it in range(IT):
                nc.sync.dma_start(kt_sb[:, it, :], kt_dv[:, it, :])

            for qt in range(QT):
                qt_t = qtpool.tile([P, IT, P], R)
                nc.sync.dma_start(qt_t[:], qt_dv[:, :, qt * P : (qt + 1) * P])

                s_sb = spool.tile([P, S], FP32)
                pmax = stpool.tile([P, KB], FP32)
                for kb in range(KB):
                    ps = pspool.tile([P, 512], FP32)
                    for it in range(IT):
                        _mm(
                            nc,
                            ps[:],
                            qt_t[:, it, :],
                            kt_sb[:, it, kb * 512 : (kb + 1) * 512],
                            start=(it == 0),
                            stop=(it == IT - 1),
                        )
                    nc.vector.reduce_max(
                        pmax[:, kb : kb + 1], ps[:], axis=mybir.AxisListType.X
                    )
                    nc.any.tensor_copy(s_sb[:, kb * 512 : (kb + 1) * 512], ps[:])

                negm = stpool.tile([P, 1], FP32)
                nc.vector.reduce_max(
                    negm[:], pmax[:], axis=mybir.AxisListType.X, negate=True
                )
                lsum = stpool.tile([P, 1], FP32)
                nc.scalar.activation(
                    s_sb[:],
                    s_sb[:],
                    mybir.ActivationFunctionType.Exp,
                    bias=negm[:],
                    accum_out=lsum[:],
                )
                nc.vector.reciprocal(rl_all[:, qt : qt + 1], lsum[:])

                at_acc = atpool.tile([P, KT, P], R)
                for kt_i in range(KT):
                    tp = tppool.tile([P, P], FP32)
                    nc.tensor.transpose(
                        tp[:], s_sb[:, kt_i * P : (kt_i + 1) * P], ident[:]
                    )
                    nc.any.tensor_copy(at_acc[:, kt_i, :], tp[:])
                nc.sync.dma_start(at_dv[:, :, qt * P : (qt + 1) * P], at_acc[:])

        # ---------- Phase 3: out = gelu((A^T.T @ V) / l) + x ----------
        with (
            tc.tile_pool(name="vres", bufs=1) as vpool,
            tc.tile_pool(name="ats", bufs=2) as atpool,
            tc.tile_pool(name="o3", bufs=2) as opool,
            tc.tile_pool(name="xq3", bufs=2) as xqpool,
            tc.tile_pool(name="ps3", bufs=4, space="PSUM") as pspool,
        ):
            v_sb = vpool.tile([P, KT, D], R)
            for g in range(8):
                nc.sync.dma_start(
                    v_sb[:, g * 4 : (g + 1) * 4, :], v_dv[:, g * 4 : (g + 1) * 4, :]
                )

            act_fn = (
                mybir.ActivationFunctionType.Gelu
                if use_gelu
                else mybir.ActivationFunctionType.Copy
            )
            for qt in range(QT):
                at_t = atpool.tile([P, KT, P], R)
                nc.sync.dma_start(at_t[:], at_dv[:, :, qt * P : (qt + 1) * P])
                xq_t = xqpool.tile([P, D], FP32)
                nc.sync.dma_start(xq_t[:], xq[qt * P : (qt + 1) * P, :])

                o_sb = opool.tile([P, D], FP32)
                for ib in range(2):
                    ps = pspool.tile([P, 512], FP32)
                    for kt_i in range(KT):
                        _mm(
                            nc,
                            ps[:],
                            at_t[:, kt_i, :],
                            v_sb[:, kt_i, ib * 512 : (ib + 1) * 512],
                            start=(kt_i == 0),
                            stop=(kt_i == KT - 1),
                        )
                    nc.scalar.activation(
                        o_sb[:, ib * 512 : (ib + 1) * 512],
                        ps[:],
                        act_fn,
                        scale=rl_all[:, qt : qt + 1],
                    )
                nc.vector.tensor_add(o_sb[:], o_sb[:], xq_t[:])
                nc.sync.dma_start(out[qt * P : (qt + 1) * P, :], o_sb[:])

    nc.compile()
    if not nc.is_finalized():
        nc.finalize()
    return nc


class _Runner:
    """SPMD runner mirroring bass2jax.run_bass_via_pjrt, but with a cached
    compiled callable so repeated calls (timing) skip recompilation."""

    def __init__(self, nc):
        import jax
        import jax.core

        self._jax = jax
        self.nc = nc

        from concourse import mybir as _mb
        from concourse.bass2jax import install_neuronx_cc_hook

        install_neuronx_cc_hook()
        assert nc.dbg_addr is None

        partition_name = (
            nc.partition_id_tensor.name if nc.partition_id_tensor else None
        )
        self.partition_name = partition_name
        in_names = []
        out_names = []
        out_avals = []
        for alloc in nc.m.functions[0].allocations:
            if not isinstance(alloc, _mb.MemoryLocationSet):
                continue
            name = alloc.memorylocations[0].name
            if alloc.kind == "ExternalInput":
                if name != partition_name:
                    in_names.append(name)
            elif alloc.kind == "ExternalOutput":
                shape = tuple(alloc.tensor_shape)
                dtype = _mb.dt.np(alloc.dtype)
                out_avals.append(jax.core.ShapedArray(shape, dtype))
                out_names.append(name)
        self.in_names = in_names
        self.out_names = out_names
        self.out_avals = out_avals
        self._compiled = None

    def _build(self):
        import jax
        import numpy as _np
        from jax.experimental.shard_map import shard_map
        from jax.sharding import Mesh, NamedSharding, PartitionSpec

        from concourse.bass2jax import _bass_exec_p, partition_id_tensor

        nc = self.nc
        in_names = list(self.in_names)
        out_names = list(self.out_names)
        out_avals = list(self.out_avals)
        all_in_names = in_names + out_names
        if self.partition_name is not None:
            all_in_names = all_in_names + [self.partition_name]
        n_params = len(in_names)
        n_outs = len(out_names)
        partition_name = self.partition_name

        def _body(*args):
            operands = list(args)
            if partition_name is not None:
                operands.append(partition_id_tensor())
            outs = _bass_exec_p.bind(
                *operands,
                out_avals=tuple(out_avals),
                in_names=tuple(all_in_names),
                out_names=tuple(out_names),
                lowering_input_output_aliases=(),
                sim_require_finite=True,
                sim_require_nnan=True,
                nc=nc,
            )
            return tuple(outs)

        devices = jax.devices()[:N_CORES]
        mesh = Mesh(_np.asarray(devices), ("core",))
        self.mesh = mesh
        self.sharding = NamedSharding(mesh, PartitionSpec("core"))
        donate = tuple(range(n_params, n_params + n_outs))
        in_specs = (PartitionSpec("core"),) * (n_params + n_outs)
        out_specs = (PartitionSpec("core"),) * n_outs
        self._compiled = jax.jit(
            shard_map(
                _body,
                mesh=mesh,
                in_specs=in_specs,
                out_specs=out_specs,
                check_rep=False,
            ),
            donate_argnums=donate,
            keep_unused=True,
        )

        def _zeros():
            import jax.numpy as jnp

            return tuple(
                jnp.zeros((N_CORES * a.shape[0], *a.shape[1:]), a.dtype)
                for a in out_avals
            )

        self._zeros_fn = jax.jit(
            _zeros, out_shardings=(self.sharding,) * n_outs
        )

    def place_inputs(self, in_maps):
        """Concatenate per-core inputs and put them on devices."""
        import jax

        if self._compiled is None:
            self._build()
        concat = [
            np.concatenate(
                [np.asarray(in_maps[c][nm]) for c in range(N_CORES)], axis=0
            )
            for nm in self.in_names
        ]
        return [jax.device_put(a, self.sharding) for a in concat]

    def run(self, dev_inputs):
        import jax

        outs = self._compiled(*dev_inputs, *self._zeros_fn())
        outs = jax.block_until_ready(outs)
        return [
            {
                nm: np.asarray(outs[i]).reshape(
                    N_CORES, *self.out_avals[i].shape
                )[c]
                for i, nm in enumerate(self.out_names)
            }
            for c in range(N_CORES)
        ]

    def time(self, dev_inputs, iters=8):
        import time as _time

        import jax

        times = []
        for _ in range(iters):
            zo = jax.block_until_ready(self._zeros_fn())
            t0 = _time.perf_counter()
            outs = self._compiled(*dev_inputs, *zo)
            jax.block_until_ready(outs)
            times.append(_time.perf_counter() - t0)
        return min(times), times


_NC_CACHE = {}


def _get_runner(use_gelu=True):
    key = (use_gelu, str(MM_DT))
    if key not in _NC_CACHE:
        _NC_CACHE[key] = _Runner(build_nc(use_gelu=use_gelu))
    return _NC_CACHE[key]


LAST_TIME_S = None


def _make_in_maps(x, Wk, Wq, Wv):
    in_maps = []
    for core in range(N_CORES):
        b, h = core // 2, core % 2
        xT_b = np.ascontiguousarray(x[b].T)
        in_maps.append(
            {
                "xT": xT_b,
                "xTq": np.ascontiguousarray(xT_b[:, h * QH : (h + 1) * QH]),
                "xq": np.ascontiguousarray(x[b, h * QH : (h + 1) * QH]),
                "wq": Wq,
                "wk": Wk,
                "wv": Wv,
            }
        )
    return in_maps


def kernel(x, Wk, Wq, Wv):
    global LAST_TIME_S

    x = np.asarray(x, dtype=np.float32)
    Wk = np.ascontiguousarray(np.asarray(Wk, dtype=np.float32))
    Wq = np.ascontiguousarray(np.asarray(Wq, dtype=np.float32))
    Wv = np.ascontiguousarray(np.asarray(Wv, dtype=np.float32))

    runner = _get_runner(use_gelu=True)
    dev_inputs = runner.place_inputs(_make_in_maps(x, Wk, Wq, Wv))
    results = runner.run(dev_inputs)

    iters = int(os.environ.get("BASS_TIME_ITERS", "0"))
    if iters > 0:
        LAST_TIME_S = runner.time(dev_inputs, iters=iters)

    out = np.empty((B, S, D), np.float32)
    for core in range(N_CORES):
        b, h = core // 2, core % 2
        out[b, h * QH : (h + 1) * QH] = results[core]["out"]
    return out
